# revision 1
# baseline (speedup 1.0000x reference)
"""ExpanderConv2d as a Bass/Tile kernel for Trainium2, data-parallel over batch
across 8 NeuronCores.

Reference op: y = conv2d(x, weight * mask), N=32, C=256->256, 56x56, k=3,
stride 1, pad 1.  Implemented per-core as 9 shifted matmuls (one per kernel
tap, x2 input-channel chunks, x2 output-channel chunks) accumulating fp32 in
PSUM over a zero-padded 58x58 input laid out channels-on-partitions.  Inputs
and weights are cast to fp16 (activations/weights only; accumulation stays
fp32) -- measured 2.8e-4 scale-relative error vs the fp32 reference, and the
fp16 weight load (FWL) keeps the PE at its 1 column/cycle streaming rate,
~189 ns per 448-column matmul.

Sharding: batch 32 -> 4 images per core; the masked weight (1.1 MB fp16) is
replicated to every core.
"""

import numpy as np

N_CORES = 8
IMG_PER_CORE = 4
C = 256
H = 56
HP = H + 2          # padded spatial edge
K = 3
RPT = 8             # output rows per PSUM tile
RG = H // RPT       # 7 row-groups
NT = RPT * H        # 448 moving-dim elements per matmul
NW = K * K * 2 * 2  # 36 weight tiles of [128ic, 128oc]


def _split_waits(nc, max_waits=1):
    """walrus in this container rejects instructions carrying more than one
    semaphore wait ("Too many sync wait commands").  Hoist the extra waits onto
    injected single-wait NoOps on the same engine just before the instruction —
    sem waits block the engine, so a chain of single waits is equivalent."""
    import concourse.mybir as mybir

    for f in nc.m.functions:
        for blk in f.blocks:
            out = []
            changed = False
            for inst in blk.instructions:
                si = inst.sync_info
                if si and si.on_wait and len(si.on_wait) > max_waits:
                    waits = list(si.on_wait)
                    extra, keep = waits[:-max_waits], waits[-max_waits:]
                    for j, w in enumerate(extra):
                        out.append(
                            mybir.InstNoOp(
                                name=f"{inst.name}-w{j}",
                                engine=inst.engine,
                                ins=[],
                                outs=[],
                                sync_info=mybir.SyncInfo(on_wait=[w], on_update=[]),
                                bass_nofuse=True,
                            )
                        )
                    si.on_wait = keep
                    changed = True
                out.append(inst)
            if changed:
                blk.instructions = out


def _build_nc():
    import concourse.bass as bass
    import concourse.mybir as mybir
    from concourse.tile import TileContext

    f32 = mybir.dt.float32
    f16 = mybir.dt.float16

    nc = bass.Bass("TRN2", target_bir_lowering=False, debug=False)
    x_d = nc.dram_tensor("x", [IMG_PER_CORE, C, H, H], f16, kind="ExternalInput").ap()
    w_d = nc.dram_tensor("w", [128, NW * 128], f16, kind="ExternalInput").ap()
    y_d = nc.dram_tensor("y", [IMG_PER_CORE, C, H, H], f32, kind="ExternalOutput").ap()

    with TileContext(nc) as tc:
        with (
            tc.tile_pool(name="wpool", bufs=1) as wp,
            tc.tile_pool(name="xpool", bufs=1) as xp,
            tc.tile_pool(name="psum", bufs=8, space="PSUM") as pp,
            tc.tile_pool(name="osb", bufs=6) as op,
        ):
            w_sb = wp.tile([128, NW * 128], f16, name="w_sb", tag="w_sb")
            # Chunked so the first group's 18 tiles (occ0) land in the first
            # two completions; the rest can trail.
            wq = 0
            for wn in (8, 10, 18):
                nc.scalar.dma_start(
                    out=w_sb[:, wq * 128 : (wq + wn) * 128],
                    in_=w_d[:, wq * 128 : (wq + wn) * 128],
                )
                wq += wn

            # Warm the PE clock gate (HAM) with throwaway matmuls on scratch
            # data while the first input/weight DMAs are still in flight --
            # otherwise the first ~3.4us of real matmuls run at 1.2 GHz.
            warm = wp.tile([128, NT], f16, name="warm", tag="warm")
            nc.vector.memset(warm[:], 0.0)
            warm_ps = pp.tile([128, NT], f32, name="ps", tag="ps")
            N_WARM = 10
            for i in range(N_WARM):
                nc.tensor.matmul(
                    warm_ps[:], warm[:, :128], warm[:], start=(i == 0), stop=(i == N_WARM - 1)
                )

            # Padded input buffers: [icc][ping/pong], borders zeroed once.
            xts = [
                [
                    xp.tile([128, HP, H], f16, name=f"xp{icc}{b}", tag=f"xp{icc}{b}")
                    for b in range(2)
                ]
                for icc in range(2)
            ]
            for b in range(2):
                for icc in range(2):
                    xt = xts[icc][b]
                    nc.vector.memset(xt[:, 0, :], 0.0)
                    nc.vector.memset(xt[:, HP - 1, :], 0.0)

            for img in range(IMG_PER_CORE):
                # Rows-only padding means the interior is contiguous per
                # partition: DMA straight into the matmul layout.  Row-blocked
                # so the first matmuls wait only on the first block.
                r0 = 0
                for rows in (9, 11, 12, 12, 12):
                    for icc in range(2):
                        xt = xts[icc][img % 2]
                        nc.sync.dma_start(
                            out=xt[:, 1 + r0 : 1 + r0 + rows, :],
                            in_=x_d[img, icc * 128 : (icc + 1) * 128, r0 : r0 + rows, :],
                        )
                    r0 += rows
                for occ in range(2):
                    for rg in range(RG):
                        ps = pp.tile([128, RPT, H], f32, name="ps", tag="ps")
                        for t, (ky, kx, icc) in enumerate(
                            (ky, kx, icc)
                            for ky in range(K)
                            for kx in range(K)
                            for icc in range(2)
                        ):
                            xt = xts[icc][img % 2]
                            widx = occ * 18 + (ky * K + kx) * 2 + icc
                            # Rows/cols that only read the zero padding add
                            # nothing to PSUM -- don't stream them.  Cells a
                            # matmul skips are first-written by a later tap
                            # (has_written accumulate-vs-overwrite semantics).
                            r_lo = 1 if (ky == 0 and rg == 0) else 0
                            r_hi = RPT - (1 if (ky == K - 1 and rg == RG - 1) else 0)
                            c_lo = 1 if kx == 0 else 0
                            c_hi = H - (1 if kx == K - 1 else 0)
                            nc.tensor.matmul(
                                ps[:, r_lo:r_hi, c_lo:c_hi],
                                w_sb[:, widx * 128 : (widx + 1) * 128],
                                xt[
                                    :,
                                    rg * RPT + ky + r_lo : rg * RPT + ky + r_hi,
                                    kx - 1 + c_lo : kx - 1 + c_hi,
                                ],
                                start=(t == 0),
                                stop=(t == 17),
                            )
                        ot = op.tile([128, RPT, H], f32, name="ot", tag="ot")
                        nc.any.tensor_copy(out=ot[:], in_=ps[:])
                        nc.scalar.dma_start(
                            out=y_d[img, occ * 128 : (occ + 1) * 128, rg * RPT : (rg + 1) * RPT, :],
                            in_=ot[:],
                        )

    _split_waits(nc)
    return nc


def _prep_weight(weight: np.ndarray, mask: np.ndarray) -> np.ndarray:
    """[OC, IC, K, K] masked weight -> [128ic, (occ,ky,kx,icc)*128oc] lhsT blocks."""
    wm = (weight * mask).astype(np.float16)
    t = wm.reshape(2, 128, 2, 128, K, K)           # [occ, oc, icc, ic, ky, kx]
    t = t.transpose(3, 0, 4, 5, 2, 1)              # [ic, occ, ky, kx, icc, oc]
    return np.ascontiguousarray(t.reshape(128, NW * 128))


def kernel(x: np.ndarray, weight: np.ndarray, mask: np.ndarray) -> np.ndarray:
    from concourse.bass_utils import run_bass_kernel_spmd

    x = np.asarray(x, dtype=np.float32)
    x16 = np.ascontiguousarray(x.astype(np.float16))
    w_host = _prep_weight(np.asarray(weight), np.asarray(mask))

    nc = _build_nc()
    in_maps = [
        {
            "x": np.ascontiguousarray(x16[c * IMG_PER_CORE : (c + 1) * IMG_PER_CORE]),
            "w": w_host,
        }
        for c in range(N_CORES)
    ]
    res = run_bass_kernel_spmd(nc, in_maps, core_ids=list(range(N_CORES)))
    out = np.empty_like(x)
    for c in range(N_CORES):
        out[c * IMG_PER_CORE : (c + 1) * IMG_PER_CORE] = res.results[c]["y"]
    return out

